# revision 14
# baseline (speedup 1.0000x reference)
"""Trainium2 Bass kernel for nn_DifferentiableRobotModel (self-collision link
distances from batched forward kinematics).

Pure data parallel over the batch (rollout) dim: 8192 rollouts -> 1024/core
on 8 NeuronCores. All FK params / sphere tables / masks are tiny and
replicated.

v2 pipeline (per core, bc = 1024 batches, 128 spheres = 16 links x 8):
  1. FK on DVE, batch-on-partitions: serial chain over 16 links using
     M_l(q) = P_l + sin(q) Q_l + (1-cos q) S_l (P,Q,S host-precomputed),
     producing per link [R^T | t] in the "S" layout. Then recenter all t by
     t_link8 (shrinks |c| so fp16 center storage stays accurate).
  2. PE transposes planes k=0..2 of S -> mall [64, 3*bc] fp16:
     row (l,m) plane k = R_l[k,m] (m<3) or t'_l[k] (m=3).
  3. One batched fp16 matmul per 128-batch tile: world centers
     c[j, (b,k)] = bd2x^T @ mall  (bd2x = block-diag [x;1] per link, fixed
     stationary). PSUM fp32 -> ctt planes 0..2 (fp16).
  4. Derived ctt planes (DVE): sq = |c|^2 in fp32, split sq = sqh + sql
     (two fp16 rows, exact to ~1e-7) so the gram's d^2 keeps fp32-level
     accuracy despite fp16 operands (no cancellation blowup); -2c planes;
     const 1 planes. T1 = [c,1,1,sqh,sql], T2 = [-2c,sqh,sql,1,1].
  5. Per 16-batch chunk: 2 PE transposes ctt -> TT [112, 256] fp16
     (rows 7k..7k+7 = T1/T2 of batch k), then 16 per-batch gram matmuls
     d2[i,j] = T1^T @ T2 (fp16, 1 cy/col) into PSUM fp32.
  6. ACT: s = sqrt(d2 + EPS) -> fp16. DVE: y = RJM - s (RJM = r_j with
     pair mask folded, -1000 for ignored); segmented max over j -> z[i,b].
  7. Tail per tile: z + r_i, PE transpose, grouped max over each link's
     8 spheres -> out [batch, 16].
"""
import sys
import numpy as np

sys.path.insert(0, "/opt/trn_rl_repo")

import concourse.bass as bass  # noqa: E402
import concourse.tile as tile  # noqa: E402
from concourse import bacc, mybir  # noqa: E402
from contextlib import ExitStack  # noqa: E402

F32 = mybir.dt.float32
F16 = mybir.dt.float16
AF = mybir.ActivationFunctionType
ALU = mybir.AluOpType
AX = mybir.AxisListType

B, L, NS = 8192, 16, 8
N = L * NS              # 128 spheres
NCORES = 8
EPS = np.float32(4e-6)  # d2 positivity shift (split-sq keeps d2 >= -O(1e-6))
MASKVAL = np.float32(-1000.0)
CHUNK = 16              # batches per transpose/gram chunk (7*16=112 rows)

_CACHE = {}


# ---------------------------------------------------------------- host consts
def _host_consts(fixed_rot, fixed_trans, joint_axes, link_spheres,
                 collision_mask, bc):
    f32, f16 = np.float32, np.float16
    ax = np.asarray(joint_axes, f32)
    K = np.zeros((L, 3, 3), f32)
    K[:, 0, 1], K[:, 0, 2] = -ax[:, 2], ax[:, 1]
    K[:, 1, 0], K[:, 1, 2] = ax[:, 2], -ax[:, 0]
    K[:, 2, 0], K[:, 2, 1] = -ax[:, 1], ax[:, 0]
    K2 = np.einsum("lij,ljk->lik", K, K).astype(f32)
    A = np.asarray(fixed_rot, f32)
    P = A
    Q = np.einsum("lij,ljk->lik", A, K).astype(f32)
    S = np.einsum("lij,ljk->lik", A, K2).astype(f32)

    # pqs [128, 432]: sections P/Q/S, col sec*144 + l*9 + 3j+k, replicated rows
    pqs = np.zeros((128, 432), f32)
    for sec, Mx in enumerate((P, Q, S)):
        pqs[:, sec * 144:(sec + 1) * 144] = Mx.reshape(1, L * 9)
    fb = np.zeros((128, 48), f32)
    fb[:, :] = np.asarray(fixed_trans, f32).reshape(1, L * 3)

    x = np.asarray(link_spheres, f32)[..., :3]           # [L,NS,3]
    r = np.asarray(link_spheres, f32)[..., 3].reshape(N)

    # bd2x [64, 128] fp16: rows (l, m): m<3 -> x[l,:,m] on link l's cols,
    # m=3 -> 1 on link l's cols
    bd2x = np.zeros((64, N), f32)
    for l in range(L):
        for k in range(3):
            bd2x[4 * l + k, l * NS:(l + 1) * NS] = x[l, :, k]
        bd2x[4 * l + 3, l * NS:(l + 1) * NS] = 1.0
    bd2x = bd2x.astype(f16)

    li = np.arange(N) // NS
    allowed = np.abs(li[:, None] - li[None, :]) > 1
    cm = np.asarray(collision_mask)[li[:, None], li[None, :]]
    allowed = allowed & cm
    rjm = np.where(allowed, r[None, :], MASKVAL).astype(f16)
    rjm8 = np.tile(rjm, (1, 8))
    rcol = r.reshape(N, 1).astype(f32)
    ident = np.eye(128, dtype=f32)
    ident16 = np.eye(128, dtype=f16)
    epsb = np.full((128, 1), EPS, f32)
    return dict(pqs=pqs, fb=fb, bd2x=bd2x, rjm=rjm, rjm8=rjm8, rcol=rcol,
                ident=ident, ident16=ident16, epsb=epsb)


# ---------------------------------------------------------------- device build
def _build_nc(nt):
    """Build + compile the per-core Bass module for nt tiles of 128 batches."""
    bc = nt * 128
    nc = bacc.Bacc("TRN2", target_bir_lowering=False, debug=False,
                   num_devices=NCORES)

    q_d = nc.dram_tensor("q", [bc, L], F32, kind="ExternalInput").ap()
    pqs_d = nc.dram_tensor("pqs", [128, 432], F32, kind="ExternalInput").ap()
    fb_d = nc.dram_tensor("fb", [128, 48], F32, kind="ExternalInput").ap()
    bd2x_d = nc.dram_tensor("bd2x", [64, N], F16, kind="ExternalInput").ap()
    rjm_d = nc.dram_tensor("rjm", [N, N], F16, kind="ExternalInput").ap()
    rjm8_d = nc.dram_tensor("rjm8", [N, 8 * N], F16, kind="ExternalInput").ap()
    rcol_d = nc.dram_tensor("rcol", [N, 1], F32, kind="ExternalInput").ap()
    ident_d = nc.dram_tensor("ident", [128, 128], F32,
                             kind="ExternalInput").ap()
    ident16_d = nc.dram_tensor("ident16", [128, 128], F16,
                               kind="ExternalInput").ap()
    epsb_d = nc.dram_tensor("epsb", [128, 1], F32, kind="ExternalInput").ap()
    out_d = nc.dram_tensor("out", [bc, L], F32, kind="ExternalOutput").ap()

    # persistent SBUF tensors
    qsb = nc.alloc_sbuf_tensor("qsb", [128, 16 * nt], F32).ap()
    sinb = nc.alloc_sbuf_tensor("sinb", [128, 16 * nt], F32).ap()
    cosb = nc.alloc_sbuf_tensor("cosb", [128, 16 * nt], F32).ap()
    omcb = nc.alloc_sbuf_tensor("omcb", [128, 16 * nt], F32).ap()
    pqs = nc.alloc_sbuf_tensor("pqs_sb", [128, 432], F32).ap()
    fbt = nc.alloc_sbuf_tensor("fb_sb", [128, 48], F32).ap()
    bd2x = nc.alloc_sbuf_tensor("bd2x_sb", [64, N], F16).ap()
    rjm = nc.alloc_sbuf_tensor("rjm_sb", [N, N], F16).ap()
    rjm8 = nc.alloc_sbuf_tensor("rjm8_sb", [N, 8 * N], F16).ap()
    rcol = nc.alloc_sbuf_tensor("rcol_sb", [N, 1], F32).ap()
    ident = nc.alloc_sbuf_tensor("ident_sb", [128, 128], F32).ap()
    ident16 = nc.alloc_sbuf_tensor("ident16_sb", [128, 128], F16).ap()
    epsb = nc.alloc_sbuf_tensor("epsb_sb", [128, 1], F32).ap()
    # FK state: col = t*816 + (slot*4 + a)*12 + b holds s[a,b]; s = R^T
    # (s[a,b] = R[b,a]), t at (3, k). slot 0 = identity pose.
    SP = 816
    sfk = nc.alloc_sbuf_tensor("sfk", [128, SP * nt], F32).ap()
    mw = nc.alloc_sbuf_tensor("mw", [128, 144 * nt], F32).ap()
    mw2 = nc.alloc_sbuf_tensor("mw2", [128, 144 * nt], F32).ap()
    tscr = nc.alloc_sbuf_tensor("tscr", [128, 9 * nt], F32).ap()
    tsc2 = nc.alloc_sbuf_tensor("tsc2", [128, 3 * nt], F32).ap()
    # mall [64, 3*bc] fp16, plane-major: col = k*bc + b
    mall = nc.alloc_sbuf_tensor("mall", [64, 3 * bc], F16).ap()
    # ctt1/ctt2 [128, 7*bc] fp16: col = b*7 + attr (contiguous per batch)
    # ctt1 (T1): 0-2 c, 3/4 one, 5 sqh, 6 sql
    # ctt2 (T2): 0-2 -2c, 3 sqh, 4 sql, 5/6 one
    ctt1 = nc.alloc_sbuf_tensor("ctt1", [128, 7 * bc], F16).ap()
    ctt2 = nc.alloc_sbuf_tensor("ctt2", [128, 7 * bc], F16).ap()
    # tta ring: 16 slots of [112, 256] (T1|T2 per chunk), halves alternate
    # per super-chunk (=tile)
    tta = nc.alloc_sbuf_tensor("tta", [112, 16 * 256], F16).ap()
    # block-diag gram moving operands: per super-chunk [112, 8*2048]
    # (zeros persist; diagonal blocks rewritten by batched scatter DMAs)
    xbd0 = nc.alloc_sbuf_tensor("xbd0", [112, 16384], F16).ap()
    xbd1 = nc.alloc_sbuf_tensor("xbd1", [112, 16384], F16).ap()

    def cap(base, offset, dims):
        """Custom AP on a persistent tensor: dims = [[step,count],...] (free)."""
        pitch = base.tensor.shape[-1]
        nparts = base.tensor.shape[0]
        return bass.AP(tensor=base.tensor, offset=offset,
                       ap=[[pitch, nparts]] + list(dims))

    def capp(base, prow, nrow, offset, dims):
        """Custom AP with partition sub-range [prow, prow+nrow)."""
        pitch = base.tensor.shape[-1]
        return bass.AP(tensor=base.tensor, offset=prow * pitch + offset,
                       ap=[[pitch, nrow]] + list(dims))

    with tile.TileContext(nc) as tc, ExitStack() as ctx:
        prepool = ctx.enter_context(tc.tile_pool(name="pre", bufs=1,
                                                 space="PSUM"))
        ttpool = ctx.enter_context(tc.tile_pool(name="ttp", bufs=2,
                                                space="PSUM"))
        grpool = ctx.enter_context(tc.tile_pool(name="gram", bufs=2,
                                                space="PSUM"))
        ttsbp = ctx.enter_context(tc.tile_pool(name="ttsb", bufs=2))
        sqwp = ctx.enter_context(tc.tile_pool(name="sqw", bufs=2))
        spool = ctx.enter_context(tc.tile_pool(name="spool", bufs=3))
        ypool = ctx.enter_context(tc.tile_pool(name="y", bufs=3))
        zpool = ctx.enter_context(tc.tile_pool(name="z", bufs=2))
        z2pool = ctx.enter_context(tc.tile_pool(name="z2", bufs=2))
        outp = ctx.enter_context(tc.tile_pool(name="outsb", bufs=2))

        # ---- input DMAs
        nc.sync.dma_start(pqs, pqs_d)
        nc.sync.dma_start(fbt, fb_d)
        nc.sync.dma_start(bd2x, bd2x_d)
        nc.sync.dma_start(rjm, rjm_d)
        nc.sync.dma_start(rjm8, rjm8_d)
        nc.sync.dma_start(rcol, rcol_d)
        nc.sync.dma_start(ident, ident_d)
        nc.sync.dma_start(ident16, ident16_d)
        nc.sync.dma_start(epsb, epsb_d)
        for t in range(nt):
            nc.sync.dma_start(cap(qsb, 16 * t, [[1, 16]]),
                              q_d[128 * t:128 * (t + 1), :])

        # ---- sin / cos / (1-cos)
        nc.scalar.activation(sinb, qsb, AF.Sin)
        # 1 - cos(q) = 2 sin^2(q/2); Sin LUT domain is [-pi, pi]
        nc.scalar.activation(cosb, qsb, AF.Sin, scale=0.5)
        nc.vector.tensor_mul(omcb, cosb, cosb)
        nc.vector.tensor_scalar_mul(omcb, omcb, 2.0)

        # ---- zero-fill: slot0 of sfk = identity pose
        nc.vector.memset(cap(sfk, 0, [[SP, nt], [1, 48]]), 0.0)
        nc.vector.memset(cap(sfk, 0, [[SP, nt], [13, 3]]), 1.0)  # I diag
        # const-1 planes
        nc.vector.memset(cap(ctt1, 3, [[7, bc], [1, 2]]), 1.0)
        nc.vector.memset(cap(ctt2, 5, [[7, bc], [1, 2]]), 1.0)
        # block-diag X zeros (written once; only diagonal blocks rewritten)
        nc.gpsimd.memset(xbd0, 0.0)
        nc.gpsimd.memset(xbd1, 0.0)

        # ---- M_l = P + sin*Q + (1-cos)*S for all links: mw[(t,l,(j,k))]
        mdims = [[144, nt], [9, L], [1, 9]]
        sdims = [[16, nt], [1, L], [0, 9]]
        nc.vector.tensor_mul(cap(mw, 0, mdims), cap(pqs, 144, [[0, nt]] + mdims[1:]),
                             cap(sinb, 0, sdims))
        nc.vector.tensor_mul(cap(mw2, 0, mdims), cap(pqs, 288, [[0, nt]] + mdims[1:]),
                             cap(omcb, 0, sdims))
        nc.vector.tensor_add(mw, mw, mw2)
        nc.vector.tensor_add(cap(mw, 0, mdims), cap(mw, 0, mdims),
                             cap(pqs, 0, [[0, nt]] + mdims[1:]))

        # ---- FK serial chain (R^T and t only)
        for l in range(L):
            sp, s_ = 48 * l, 48 * (l + 1)     # prev slot, this slot
            outR = cap(sfk, s_, [[SP, nt], [12, 3], [1, 3]])
            tmpR = cap(tscr, 0, [[9, nt], [3, 3], [1, 3]])
            for j in range(3):
                i0 = cap(sfk, sp + 12 * j, [[SP, nt], [0, 3], [1, 3]])
                i1 = cap(mw, 9 * l + 3 * j, [[144, nt], [1, 3], [0, 3]])
                if j == 0:
                    nc.vector.tensor_mul(outR, i0, i1)
                else:
                    nc.vector.tensor_mul(tmpR, i0, i1)
                    nc.vector.tensor_add(outR, outR, tmpR)
            # t_l = t_p + Rp @ ftrans_l
            nc.vector.tensor_mul(cap(tscr, 0, [[9, nt], [3, 3], [1, 3]]),
                                 cap(sfk, sp, [[SP, nt], [1, 3], [12, 3]]),
                                 cap(fbt, 3 * l, [[0, nt], [0, 3], [1, 3]]))
            nc.vector.reduce_sum(cap(tsc2, 0, [[3, nt], [1, 3]]),
                                 cap(tscr, 0, [[9, nt], [3, 3], [1, 3]]),
                                 axis=AX.X)
            nc.vector.tensor_add(cap(sfk, s_ + 36, [[SP, nt], [1, 3]]),
                                 cap(sfk, sp + 36, [[SP, nt], [1, 3]]),
                                 cap(tsc2, 0, [[3, nt], [1, 3]]))

        # ---- recenter: t'_l = t_l - t_link8 (slot 9)
        nc.vector.tensor_copy(cap(tsc2, 0, [[3, nt], [1, 3]]),
                              cap(sfk, 48 * 9 + 36, [[SP, nt], [1, 3]]))
        nc.vector.tensor_sub(cap(sfk, 48 + 36, [[SP, nt], [48, L], [1, 3]]),
                             cap(sfk, 48 + 36, [[SP, nt], [48, L], [1, 3]]),
                             cap(tsc2, 0, [[3, nt], [0, L], [1, 3]]))

        # ---- per tile: S planes k=0..2 -> mall; CT matmul -> ctt planes 0..2
        for t in range(nt):
            trm = prepool.tile([64, 384], F32, tag="pre")
            for k in range(3):
                nc.tensor.transpose(
                    trm[:, 128 * k:128 * (k + 1)],
                    cap(sfk, SP * t + 48 + k, [[48, 16], [12, 4]]),
                    ident)
            # mall batch-major: col = b*3 + k  (trm col = k*128 + b)
            nc.scalar.copy(
                capp(mall, 0, 64, 3 * 128 * t, [[1, 3], [3, 128]]),
                trm[:, :])
            # world centers: ctp[j, (b,k)] = bd2x^T @ mall
            ctp = prepool.tile([128, 384], F32, tag="pre")
            nc.tensor.matmul(
                ctp[:, :],
                bd2x[0:64, :],
                capp(mall, 0, 64, 3 * 128 * t, [[1, 384]]))
            nc.scalar.copy(
                cap(ctt1, 7 * 128 * t, [[7, 128], [1, 3]]),
                ctp[:, :])
            # derived planes
            c_ap = cap(ctt1, 7 * 128 * t, [[7, 128], [1, 3]])
            sqw = sqwp.tile([128, 384], F32)
            nc.vector.tensor_mul(sqw[:, :], c_ap, c_ap)
            sq32 = sqwp.tile([128, 128], F32)
            nc.vector.reduce_sum(
                sq32[:, :], sqw[:, :].rearrange("p (b k) -> p b k", k=3),
                axis=AX.X)
            # sqh (fp16) and sql = sq - sqh
            nc.gpsimd.tensor_copy(cap(ctt1, 7 * 128 * t + 5, [[7, 128]]),
                                  sq32[:, :])
            nc.vector.tensor_sub(cap(ctt1, 7 * 128 * t + 6, [[7, 128]]),
                                 sq32[:, :],
                                 cap(ctt1, 7 * 128 * t + 5, [[7, 128]]))
            nc.gpsimd.tensor_copy(
                cap(ctt2, 7 * 128 * t + 3, [[7, 128], [1, 2]]),
                cap(ctt1, 7 * 128 * t + 5, [[7, 128], [1, 2]]))
            nc.vector.tensor_scalar_mul(
                cap(ctt2, 7 * 128 * t + 0, [[7, 128], [1, 3]]),
                cap(ctt1, 7 * 128 * t + 0, [[7, 128], [1, 3]]), -2.0)

        # ---- main loop: super-chunks of 128 batches (8 chunks of 16)
        for t in range(nt):
            half = t % 2
            z = zpool.tile([128, 128], F16)
            # transposes into the tta ring half
            for cc in range(8):
                c = 8 * t + cc
                slot = 256 * (8 * half + cc)
                tt = ttpool.tile([112, 256], F16)
                nc.tensor.transpose(
                    tt[:, 0:128],
                    cap(ctt1, 7 * CHUNK * c, [[1, 112]]),
                    ident16)
                nc.tensor.transpose(
                    tt[:, 128:256],
                    cap(ctt2, 7 * CHUNK * c, [[1, 112]]),
                    ident16)
                nc.scalar.copy(capp(tta, 0, 112, slot, [[1, 256]]), tt[:, :])
            # batched scatter: 16 DMAs, each moves block k of all 8 chunks
            xbd = xbd0 if half == 0 else xbd1
            tpitch = tta.tensor.shape[-1]
            xpitch = xbd.tensor.shape[-1]
            for k in range(CHUNK):
                src_ap = bass.AP(
                    tensor=tta.tensor,
                    offset=7 * k * tpitch + 256 * 8 * half + 128,
                    ap=[[tpitch, 7], [256, 8], [1, 128]])
                dst_ap = bass.AP(
                    tensor=xbd.tensor,
                    offset=7 * k * xpitch + 128 * k,
                    ap=[[xpitch, 7], [2048, 8], [1, 128]])
                nc.sync.dma_start(dst_ap, src_ap)
            # grams + sqrt + sub + reduce per chunk
            for cc in range(8):
                slot = 256 * (8 * half + cc)
                for h in range(2):
                    gr = grpool.tile([128, 1024], F32)
                    for g in range(2):
                        nc.tensor.matmul(
                            gr[:, 512 * g:512 * (g + 1)],
                            capp(tta, 0, 112, slot, [[1, 128]]),
                            capp(xbd, 0, 112,
                                 2048 * cc + 512 * (2 * h + g), [[1, 512]]))
                    s_t = spool.tile([128, 1024], F16)
                    nc.scalar.activation(s_t[:, :], gr[:, :], AF.Sqrt,
                                         bias=epsb[0:128, 0:1])
                    y_t = ypool.tile([128, 1024], F16)
                    if (2 * cc + h) % 3 == 0:
                        nc.vector.tensor_sub(
                            y_t[:, :],
                            cap(rjm, 0, [[0, 8], [1, 128]]),
                            s_t[:, :])
                    else:
                        nc.gpsimd.tensor_sub(
                            y_t[:, :],
                            rjm8[0:128, 0:1024],
                            s_t[:, :])
                    gb0 = cc * CHUNK + 8 * h
                    nc.vector.tensor_reduce(
                        z[:, gb0:gb0 + 8],
                        y_t[:, :].rearrange("p (g j) -> p g j", j=128),
                        axis=AX.X, op=ALU.max)
            # tail for this tile
            z2 = z2pool.tile([128, 128], F16)
            nc.vector.tensor_scalar_add(z2[:, :], z[:, :],
                                        rcol[0:128, 0:1])
            ztr = prepool.tile([128, 128], F16, tag="pre")
            nc.tensor.transpose(ztr[:, :], z2[:, :], ident16)
            osb = outp.tile([128, L], F32)
            nc.vector.tensor_reduce(
                osb[:, :], ztr[:, :].rearrange("p (a b) -> p a b", a=L),
                axis=AX.X, op=ALU.max)
            nc.sync.dma_start(out_d[128 * t:128 * (t + 1), :], osb[:, :])

    nc.compile()
    return nc


def get_nc(nt):
    key = ("nc", nt)
    if key not in _CACHE:
        _CACHE[key] = _build_nc(nt)
    return _CACHE[key]


# ---------------------------------------------------------------- entry point
def kernel(q, fixed_rot, fixed_trans, joint_axes, link_spheres,
           collision_mask):
    from concourse.bass_utils import run_bass_kernel_spmd

    q = np.asarray(q, np.float32)
    bc = B // NCORES
    nt = bc // 128
    consts = _host_consts(fixed_rot, fixed_trans, joint_axes, link_spheres,
                          collision_mask, bc)
    nc = get_nc(nt)
    in_maps = []
    for c in range(NCORES):
        m = {"q": np.ascontiguousarray(q[c * bc:(c + 1) * bc]),
             "pqs": consts["pqs"], "fb": consts["fb"],
             "bd2x": consts["bd2x"], "rjm": consts["rjm"],
             "rjm8": consts["rjm8"],
             "rcol": consts["rcol"], "ident": consts["ident"],
             "ident16": consts["ident16"], "epsb": consts["epsb"]}
        in_maps.append(m)
    res = run_bass_kernel_spmd(nc, in_maps, list(range(NCORES)))
    out = np.concatenate([res.results[c]["out"] for c in range(NCORES)],
                         axis=0)
    return out.astype(np.float32)
